# revision 4
# baseline (speedup 1.0000x reference)
"""Multi-head attention (COAMultiHeadAttention) on 8 Trainium2 NeuronCores.

Sharding: batch x head-group. Core c (0..7) handles batch b = c//4 and head
group g = c%4 (4 of 16 heads, i.e. a 256-wide slice of the 1024-dim model).

Per-core schedule (v2 - row-tiled attention):
  Phase A: q/k/v projections in bf16 (d-chunk-outer PSUM accumulation),
    evac with bias add on DVE. K^T/Q^T are then DUPLICATED into both
    64-partition halves per head (SBUF->SBUF DMA) so phase B can run the
    PE as two independent 64x128 row tiles.
  Phase B: per (head, q-half) block, entirely in 64-row PE mode (no
    mode switches - mixing 64/128-row matmuls costs ~370ns per switch):
      - QK^T for key chunks i, i+1 run CONCURRENTLY on row tiles T0/T8
        (measured 245ns per pair vs 430ns serial).
      - exp is split across ScalarE (table exp) and DVE (one-instruction
        Schraudolph exp: bits16 = x*a+b written to an int16 view, bitcast
        bf16). DVE share ~6/16 keeps the approx error ~1% (softmax
        denominators use the same approximated probs, so the mean error
        cancels).
      - P~V with an extra ones column runs as key-half pairs on T0/T8
        into separate PSUM accumulators att_lo/att_hi (a single PSUM
        accumulation group must stay on one row tile - crossing tiles
        hangs the device).
      - Tail: att_lo+att_hi merge (DVE), softmax denominators from the
        ones column, reciprocal broadcast via DRAM bounce, normalize.
  Phase C: output projection partials in 128-row mode, evac alternating
    ScalarE/DVE. Host sums the 4 partials per batch in fp32 and adds bo.
"""

import os

import ml_dtypes
import numpy as np

import concourse.bass as bass  # noqa: F401  (AP types resolve through this import)
import concourse.mybir as mybir
import concourse.tile as tile
from concourse import bacc, bass_utils

F32 = mybir.dt.float32
BF16 = mybir.dt.bfloat16
I16 = mybir.dt.int16
AT = mybir.ActivationFunctionType
ALU = mybir.AluOpType

B = 2
T = 2048
D = 1024
N_HEADS = 16
HEAD_DIM = 64
N_CORES = 8
S = 256            # per-core slice of the model dim (4 heads)
NHL = 4            # heads per core
P = 128
DC = D // P        # 8 contraction chunks for the projections
TC = T // P        # 16 token chunks
QH = 1024          # q-columns per attention block
SCALE = 1.0 / np.sqrt(HEAD_DIM)

LOG2E = float(np.log2(np.e))
C_SCH = 0.0579
A_SCH = float(SCALE * 128.0 * LOG2E)
B_SCH = float(127.0 * 128.0 - 128.0 * C_SCH)
# key chunks whose exp runs on DVE (Schraudolph); rest on ScalarE
DVE_CHUNKS = (3, 5, 7, 11, 13, 15)

_CACHE = {}
LAST_STATS = {}


def _build_program():
    nc = bacc.Bacc("TRN2", target_bir_lowering=False, debug=False)

    xq_d = nc.dram_tensor("xq", [P, DC, T], BF16, kind="ExternalInput").ap()
    xk_d = nc.dram_tensor("xk", [P, DC, T], BF16, kind="ExternalInput").ap()
    xv_d = nc.dram_tensor("xv", [P, DC, T], BF16, kind="ExternalInput").ap()
    wqt_d = nc.dram_tensor("wqt", [P, DC, S], BF16, kind="ExternalInput").ap()
    wkt_d = nc.dram_tensor("wkt", [P, DC, S], BF16, kind="ExternalInput").ap()
    wvt_d = nc.dram_tensor("wvt", [P, DC, S], BF16, kind="ExternalInput").ap()
    bq_d = nc.dram_tensor("bq", [P, 2], F32, kind="ExternalInput").ap()
    bk_d = nc.dram_tensor("bk", [P, 2], F32, kind="ExternalInput").ap()
    bv_d = nc.dram_tensor("bv", [P, NHL, HEAD_DIM], F32, kind="ExternalInput").ap()
    wot_d = nc.dram_tensor("wot", [P, 2, D], BF16, kind="ExternalInput").ap()
    out_d = nc.dram_tensor("out_part", [TC, P, D], BF16, kind="ExternalOutput").ap()
    sums_d = nc.dram_tensor("sums_scr", [NHL, T], F32).ap()
    rsums_d = nc.dram_tensor("rsums_scr", [NHL, T], F32).ap()

    with tile.TileContext(nc) as tc:
        _body(tc, xq_d, xk_d, xv_d, wqt_d, wkt_d, wvt_d,
              bq_d, bk_d, bv_d, wot_d, out_d, sums_d, rsums_d)
    nc.compile()
    return nc


def _body(tc, xq_d, xk_d, xv_d, wqt_d, wkt_d, wvt_d, bq_d, bk_d, bv_d, wot_d,
          out_d, sums_d, rsums_d):
    nc = tc.nc

    from contextlib import ExitStack
    with ExitStack() as ctx:
        pers = ctx.enter_context(tc.tile_pool(name="pers", bufs=1))
        # per-head K^T/Q^T duplicated into both 64-partition halves
        kt2 = pers.tile([P, NHL, T], BF16, tag="kt2")
        qt2 = pers.tile([P, NHL, T], BF16, tag="qt2")
        v_sb = pers.tile([P, TC, NHL, 68], BF16, tag="v")
        attn_sb = pers.tile([P, 2, T], BF16, tag="attn")
        wot_sb = pers.tile([P, 2, D], BF16, tag="wot")
        bq_sb = pers.tile([P, 2], F32, tag="bq")
        bk_sb = pers.tile([P, 2], F32, tag="bk")
        bv_sb = pers.tile([P, NHL, HEAD_DIM], F32, tag="bv")
        zero_sb = pers.tile([P, 1], F32, tag="zero")
        scr_sb = pers.tile([P, 1], F32, tag="scr")

        nc.sync.dma_start(wot_sb[:], wot_d[:])
        nc.sync.dma_start(bq_sb[:], bq_d[:])
        nc.sync.dma_start(bk_sb[:], bk_d[:])
        nc.sync.dma_start(bv_sb[:], bv_d[:])
        nc.vector.memset(zero_sb[:], 0.0)
        # Preload the exp table set (~1.3us) before the attention pipeline.
        nc.scalar.activation(scr_sb[:], zero_sb[:], AT.Exp,
                             bias=zero_sb[:, 0:1], scale=1.0)
        # ones column for the P~V sum trick (col 64 of every (tchunk, head))
        nc.vector.memset(v_sb[:, :, :, 64:65], 1.0)

        # ---------------- Phase A: projections ----------------
        with tc.tile_pool(name="xw", bufs=1) as xw, \
             tc.tile_pool(name="kqsb", bufs=1) as kqsb, \
             tc.tile_pool(name="pjps", bufs=4, space="PSUM") as pjps:
            wq_sb = xw.tile([P, DC, S], BF16, tag="wq")
            wk_sb = xw.tile([P, DC, S], BF16, tag="wk")
            wv_sb = xw.tile([P, DC, S], BF16, tag="wv")
            xq_sb = xw.tile([P, DC, T], BF16, tag="xq")
            xk_sb = xw.tile([P, DC, T], BF16, tag="xk")
            xv_sb = xw.tile([P, DC, T], BF16, tag="xv")
            kt_sb = kqsb.tile([P, 2, T], BF16, tag="kt")
            qt_sb = kqsb.tile([P, 2, T], BF16, tag="qt")

            for c in range(DC):
                nc.sync.dma_start(wk_sb[:, c], wkt_d[:, c])
                nc.sync.dma_start(xk_sb[:, c], xk_d[:, c])
            for c in range(DC):
                nc.sync.dma_start(wq_sb[:, c], wqt_d[:, c])
                nc.sync.dma_start(xq_sb[:, c], xq_d[:, c])
            for c in range(DC):
                nc.sync.dma_start(wv_sb[:, c], wvt_d[:, c])
                nc.sync.dma_start(xv_sb[:, c], xv_d[:, c])

            # K^T / Q^T projections: (256, T) d-major, bf16 + bias.
            def proj_kq(x_sb, w_sb, b_sb, dst, m):
                for n in range(4):
                    ps = pjps.tile([P, 512], F32, tag="pj")
                    for d8 in range(DC):
                        nc.tensor.matmul(
                            ps[:],
                            lhsT=w_sb[:, d8, m * P:(m + 1) * P],
                            rhs=x_sb[:, d8, n * 512:(n + 1) * 512],
                            start=(d8 == 0), stop=(d8 == DC - 1))
                    nc.vector.tensor_scalar(
                        dst[:, m, n * 512:(n + 1) * 512], ps[:],
                        b_sb[:, m:m + 1], None, op0=ALU.add)

            def dup_heads(src, dst, mh):
                # duplicate each head's 64 dims into both partition halves
                for hb in range(2):
                    h = 2 * mh + hb
                    s = src[hb * 64:(hb + 1) * 64, mh, :]
                    nc.sync.dma_start(dst[0:64, h, :], s)
                    nc.sync.dma_start(dst[64:128, h, :], s)

            proj_kq(xk_sb, wk_sb, bk_sb, kt_sb, 0)
            proj_kq(xq_sb, wq_sb, bq_sb, qt_sb, 0)
            dup_heads(kt_sb, kt2, 0)
            dup_heads(qt_sb, qt2, 0)
            proj_kq(xk_sb, wk_sb, bk_sb, kt_sb, 1)
            proj_kq(xq_sb, wq_sb, bq_sb, qt_sb, 1)
            dup_heads(kt_sb, kt2, 1)
            dup_heads(qt_sb, qt2, 1)

            # V projection: token-major (T, 256) bf16 + bias
            for t16 in range(TC):
                ps = pjps.tile([P, S], F32, tag="pj")
                for d8 in range(DC):
                    nc.tensor.matmul(
                        ps[:],
                        lhsT=xv_sb[:, d8, t16 * P:(t16 + 1) * P],
                        rhs=wv_sb[:, d8, :],
                        start=(d8 == 0), stop=(d8 == DC - 1))
                nc.vector.tensor_tensor(
                    v_sb[:, t16, :, 0:64],
                    ps[:].rearrange("p (h x) -> p h x", h=NHL),
                    bv_sb[:], op=ALU.add)

        # ---------------- Phase B: attention ----------------
        # 8 blocks (head, q-half), entirely in 64-row PE mode. Key chunks
        # are processed in pairs (i, i+1): chunk i on row tile T0
        # (partitions 0-63), chunk i+1 on T8 (64-127); consecutive
        # instructions always alternate tiles so LDWEIGHTS pulls ahead and
        # the two streams run concurrently.
        with tc.tile_pool(name="stp", bufs=2, space="PSUM") as stp, \
             tc.tile_pool(name="attlp", bufs=1, space="PSUM") as attlp, \
             tc.tile_pool(name="atthp", bufs=1, space="PSUM") as atthp, \
             tc.tile_pool(name="ptp", bufs=6) as ptp, \
             tc.tile_pool(name="mrg", bufs=2) as mrgp, \
             tc.tile_pool(name="brd", bufs=2) as brdp, \
             tc.tile_pool(name="rcp", bufs=4) as rcpp:
            pending_pv = []

            def emit_pv(ent):
                att_lo, att_hi, h, i, pt = ent
                first = (i == 0)
                last = (i == TC - 1)
                for n in range(2):
                    ns = slice(n * 512, (n + 1) * 512)
                    nc.tensor.matmul(
                        att_lo[:, ns], lhsT=v_sb[0:64, i, h, 0:65],
                        rhs=pt[0:64, ns], start=first, stop=last)
                    nc.tensor.matmul(
                        att_hi[:, ns], lhsT=v_sb[64:128, i, h, 0:65],
                        rhs=pt[64:128, ns], start=first, stop=last)

            def emit_block_tail(ent):
                att_lo, att_hi, h, jh = ent
                q0 = jh * QH
                mh, hb = divmod(h, 2)
                ph = hb * 64
                # merge key-halves (also frees both PSUM accumulators);
                # only one tensor_tensor input may come from PSUM, so
                # copy att_lo out first.
                attm = mrgp.tile([65, QH], F32, tag="attm")
                nc.vector.tensor_copy(attm[:], att_lo[:])
                nc.vector.tensor_tensor(attm[:], attm[:], att_hi[:],
                                        op=ALU.add)
                # softmax denominators -> reciprocal via DRAM bounce
                # (partition-major reciprocal, then partition broadcast)
                nc.sync.dma_start(sums_d[h:h + 1, q0:q0 + QH],
                                  attm[64:65, :])
                sp = rcpp.tile([P, QH // P], F32, tag="sp")
                nc.sync.dma_start(
                    sp[:], sums_d[h, q0:q0 + QH].rearrange(
                        "(p f) -> p f", p=P))
                rp = rcpp.tile([P, QH // P], F32, tag="rp")
                nc.vector.reciprocal(rp[:], sp[:])
                nc.sync.dma_start(
                    rsums_d[h, q0:q0 + QH].rearrange("(p f) -> p f", p=P),
                    rp[:])
                rc = brdp.tile([64, QH], F32, tag="rc")
                nc.sync.dma_start(
                    rc[:], rsums_d[h:h + 1, q0:q0 + QH].broadcast_to((64, QH)))
                # normalize on the (otherwise idle) Pool engine - all SBUF
                nc.gpsimd.tensor_tensor(
                    attn_sb[ph:ph + 64, mh, q0:q0 + QH],
                    attm[0:64, :], rc[:], op=ALU.mult)

            pending_tail = None
            blocks = [(h, jh) for h in range(NHL) for jh in range(2)]
            for bi, (h, jh) in enumerate(blocks):
                q0 = jh * QH
                att_lo = attlp.tile([65, QH], F32, tag="attl", name="att_lo")
                att_hi = atthp.tile([65, QH], F32, tag="atth", name="att_hi")
                for ip in range(TC // 2):
                    i0, i1 = 2 * ip, 2 * ip + 1
                    st0 = stp.tile([P, QH], F32, tag="st", name="st0")
                    st1 = stp.tile([P, QH], F32, tag="st", name="st1")
                    # interleave T0/T8 so pairs run concurrently
                    for n in range(2):
                        ns = slice(n * 512, (n + 1) * 512)
                        qs = slice(q0 + n * 512, q0 + (n + 1) * 512)
                        nc.tensor.matmul(
                            st0[:, ns],
                            lhsT=kt2[0:64, h, i0 * P:(i0 + 1) * P],
                            rhs=qt2[0:64, h, qs],
                            start=True, stop=True)
                        nc.tensor.matmul(
                            st1[:, ns],
                            lhsT=kt2[64:128, h, i1 * P:(i1 + 1) * P],
                            rhs=qt2[64:128, h, qs],
                            start=True, stop=True)
                    if bi == 0 and ip == 0:
                        # keep-warm (64-row shaped): bridge the exp
                        # pipeline-fill so HAM never sees a >3.4us PE idle.
                        for wmm in range(6):
                            nc.tensor.matmul(
                                att_lo[:, 0:512],
                                lhsT=v_sb[0:64, 0, h, 0:65],
                                rhs=qt2[0:64, h, 0:512],
                                start=True, stop=True)
                            nc.tensor.matmul(
                                att_hi[:, 0:512],
                                lhsT=v_sb[64:128, 0, h, 0:65],
                                rhs=qt2[64:128, h, 0:512],
                                start=True, stop=True)
                    for i, st in ((i0, st0), (i1, st1)):
                        pt = ptp.tile([P, QH], BF16, tag="pt")
                        if i in DVE_CHUNKS:
                            nc.vector.tensor_scalar(
                                pt[:].bitcast(I16), st[:], A_SCH, B_SCH,
                                op0=ALU.mult, op1=ALU.add)
                        else:
                            nc.scalar.activation(pt[:], st[:], AT.Exp,
                                                 bias=zero_sb[:, 0:1],
                                                 scale=float(SCALE))
                        pending_pv.append((att_lo, att_hi, h, i, pt))
                    while len(pending_pv) > 2:
                        emit_pv(pending_pv.pop(0))
                    if ip == 1 and pending_tail is not None:
                        emit_block_tail(pending_tail)
                        pending_tail = None
                pending_tail = (att_lo, att_hi, h, jh)
            while pending_pv:
                emit_pv(pending_pv.pop(0))
            emit_block_tail(pending_tail)

        # ---------------- Phase C: output projection (partial) ----------------
        with tc.tile_pool(name="ops", bufs=3, space="PSUM") as ops, \
             tc.tile_pool(name="owm", bufs=1, space="PSUM") as owm, \
             tc.tile_pool(name="osb", bufs=4) as osb:
            wmt = owm.tile([P, 512], F32, tag="wmt")
            for m in range(TC):
                po = ops.tile([P, D], F32, tag="po")
                # keep-warm matmul: evac-paced pipeline has PE gaps
                nc.tensor.matmul(wmt[:], lhsT=attn_sb[:, 0, 0:P],
                                 rhs=wot_sb[:, 0, 0:512], start=True, stop=True)
                for sc in range(2):
                    for n in range(2):
                        nc.tensor.matmul(
                            po[:, n * 512:(n + 1) * 512],
                            lhsT=attn_sb[:, sc, m * P:(m + 1) * P],
                            rhs=wot_sb[:, sc, n * 512:(n + 1) * 512],
                            start=(sc == 0), stop=(sc == 1))
                ob = osb.tile([P, D], BF16, tag="ob")
                if m % 2 == 0:
                    nc.scalar.copy(ob[:], po[:])
                else:
                    nc.vector.tensor_copy(ob[:], po[:])
                nc.sync.dma_start(out_d[m], ob[:])


def _shard_inputs(query, key, value, wq, bq, wk, bk, wv, bv, wo):
    """Build the 8 per-core input maps (all host-side numpy)."""
    bf16 = ml_dtypes.bfloat16
    in_maps = []

    def fold_dmajor(a_t, inner):
        # (D, inner) -> [P, DC, inner]
        return np.ascontiguousarray(
            a_t.reshape(DC, P, inner).transpose(1, 0, 2))

    xs = {}
    for b in range(B):
        for name, x in (("xq", query), ("xk", key), ("xv", value)):
            xt = np.ascontiguousarray(x[b].T).astype(bf16)  # (D, T)
            xs[(name, b)] = fold_dmajor(xt, T)

    for c in range(N_CORES):
        b, g = divmod(c, NHL)
        gs = g * S
        wq_g = wq[gs:gs + S]          # (S, D)
        wk_g = wk[gs:gs + S]
        wv_g = wv[gs:gs + S]
        wo_g = wo[:, gs:gs + S]       # (D, S)
        m = {
            "xq": xs[("xq", b)],
            "xk": xs[("xk", b)],
            "xv": xs[("xv", b)],
            "wqt": fold_dmajor(np.ascontiguousarray(wq_g.T).astype(bf16), S),
            "wkt": fold_dmajor(np.ascontiguousarray(wk_g.T).astype(bf16), S),
            "wvt": fold_dmajor(np.ascontiguousarray(wv_g.T).astype(bf16), S),
            "bq": np.ascontiguousarray(
                bq[gs:gs + S].reshape(2, P).T).astype(np.float32),
            "bk": np.ascontiguousarray(
                bk[gs:gs + S].reshape(2, P).T).astype(np.float32),
            "bv": np.ascontiguousarray(np.broadcast_to(
                bv[gs:gs + S].reshape(NHL, HEAD_DIM), (P, NHL, HEAD_DIM))
            ).astype(np.float32),
            "wot": np.ascontiguousarray(
                wo_g.T.reshape(2, P, D).transpose(1, 0, 2)).astype(bf16),
        }
        in_maps.append(m)
    return in_maps


def _reference_numpy(query, key, value, mask, wq, bq, wk, bk, wv, bv, wo, bo):
    """Pure-numpy fallback for non-trivial masks (never hit for spec inputs)."""
    def lin(x, w, b):
        return np.einsum("btd,od->bto", x, w) + b
    Bq, Tq, _ = query.shape
    Q = lin(query, wq, bq).reshape(Bq, Tq, N_HEADS, HEAD_DIM).transpose(0, 2, 1, 3)
    K = lin(key, wk, bk).reshape(Bq, Tq, N_HEADS, HEAD_DIM).transpose(0, 2, 1, 3)
    V = lin(value, wv, bv).reshape(Bq, Tq, N_HEADS, HEAD_DIM).transpose(0, 2, 1, 3)
    scores = np.einsum("bhqd,bhkd->bhqk", Q, K) * SCALE
    scores = np.where(mask[:, None, :, :] == 0, -np.inf, scores)
    scores = scores - scores.max(axis=-1, keepdims=True)
    e = np.exp(scores)
    probs = e / e.sum(axis=-1, keepdims=True)
    att = np.einsum("bhqk,bhkd->bhqd", probs, V)
    att = att.transpose(0, 2, 1, 3).reshape(Bq, Tq, N_HEADS * HEAD_DIM)
    return (np.einsum("btd,od->bto", att, wo) + bo).astype(np.float32)


def _enable_local_tracing():
    """Make bass_utils' axon NTFF-trace path work in this container."""
    import sys
    import types
    try:
        import antenv.axon_hooks  # noqa: F401
    except Exception:
        try:
            from trn_agent_boot.trn_boot import _ntff_profile_via_ctypes
            hook = _ntff_profile_via_ctypes("/opt/axon/libaxon_pjrt.so")
            if hook is None:
                return False
            holder = {"hook": hook}
            m2 = types.ModuleType("antenv.axon_hooks")
            m2.get_axon_ntff_profile_hook = lambda: holder["hook"]
            m2.set_axon_ntff_profile_hook = lambda h: holder.update(hook=h)
            if "antenv" not in sys.modules:
                m1 = types.ModuleType("antenv")
                m1.axon_hooks = m2
                sys.modules["antenv"] = m1
            else:
                sys.modules["antenv"].axon_hooks = m2
            sys.modules["antenv.axon_hooks"] = m2
        except Exception:
            return False
    bass_utils.upload_artifacts = lambda tmpdir: tmpdir
    return True


def kernel(query, key, value, mask, wq, bq, wk, bk, wv, bv, wo, bo):
    query = np.asarray(query, np.float32)
    key = np.asarray(key, np.float32)
    value = np.asarray(value, np.float32)
    wq_, bq_ = np.asarray(wq, np.float32), np.asarray(bq, np.float32)
    wk_, bk_ = np.asarray(wk, np.float32), np.asarray(bk, np.float32)
    wv_, bv_ = np.asarray(wv, np.float32), np.asarray(bv, np.float32)
    wo_, bo_ = np.asarray(wo, np.float32), np.asarray(bo, np.float32)
    mask_np = np.asarray(mask)

    if not np.all(mask_np != 0):
        # Spec inputs always have an all-ones mask; keep a correct fallback.
        return _reference_numpy(query, key, value, mask_np, wq_, bq_,
                                wk_, bk_, wv_, bv_, wo_, bo_)

    if "prog" not in _CACHE:
        _CACHE["prog"] = _build_program()
    nc = _CACHE["prog"]

    in_maps = _shard_inputs(query, key, value, wq_, bq_, wk_, bk_, wv_, bv_, wo_)

    trace = os.environ.get("KERNEL_TRACE", "0") == "1"
    kw = {}
    if trace:
        trace = _enable_local_tracing()
        if trace:
            tdir = os.environ.get("KERNEL_TRACE_DIR")
            if tdir:
                os.makedirs(tdir, exist_ok=True)
                kw["tmpdir"] = tdir
    try:
        res = bass_utils.run_bass_kernel_spmd(
            nc, in_maps, core_ids=list(range(N_CORES)), trace=trace, **kw)
    except Exception:
        if not trace:
            raise
        import traceback
        traceback.print_exc()
        res = bass_utils.run_bass_kernel_spmd(
            nc, in_maps, core_ids=list(range(N_CORES)), trace=False)

    LAST_STATS.clear()
    LAST_STATS["exec_time_ns"] = res.exec_time_ns
    LAST_STATS["profile_json"] = res.profile_json
    if res.instructions_and_trace is not None:
        LAST_STATS["trace_url"] = res.instructions_and_trace[1]

    out = np.empty((B, T, D), np.float32)
    for b in range(B):
        acc = np.zeros((T, D), np.float32)
        for g in range(NHL):
            acc += res.results[b * NHL + g]["out_part"].reshape(T, D).astype(
                np.float32)
        out[b] = acc + bo_
    return out


# revision 14
# speedup vs baseline: 1.1355x; 1.1355x over previous
"""Multi-head attention (COAMultiHeadAttention) on 8 Trainium2 NeuronCores.

Sharding: batch x head-group. Core c (0..7) handles batch b = c//4 and head
group g = c%4 (4 of 16 heads, i.e. a 256-wide slice of the 1024-dim model).

Per-core schedule (v2 - row-tiled attention):
  Phase A: q/k/v projections in bf16 (d-chunk-outer PSUM accumulation),
    evac with bias add on DVE. K^T/Q^T are then DUPLICATED into both
    64-partition halves per head (SBUF->SBUF DMA) so phase B can run the
    PE as two independent 64x128 row tiles.
  Phase B: per (head, q-half) block, entirely in 64-row PE mode (no
    mode switches - mixing 64/128-row matmuls costs ~370ns per switch):
      - QK^T for key chunks i, i+1 run CONCURRENTLY on row tiles T0/T8
        (measured 245ns per pair vs 430ns serial).
      - exp is split across ScalarE (table exp) and DVE (one-instruction
        Schraudolph exp: bits16 = x*a+b written to an int16 view, bitcast
        bf16). DVE share ~6/16 keeps the approx error ~1% (softmax
        denominators use the same approximated probs, so the mean error
        cancels).
      - P~V with an extra ones column runs as key-half pairs on T0/T8
        into separate PSUM accumulators att_lo/att_hi (a single PSUM
        accumulation group must stay on one row tile - crossing tiles
        hangs the device).
      - Tail: att_lo+att_hi merge (DVE), softmax denominators from the
        ones column, reciprocal broadcast via DRAM bounce, normalize.
  Phase C: output projection partials in 128-row mode, evac alternating
    ScalarE/DVE. Host sums the 4 partials per batch in fp32 and adds bo.
"""

import os

import ml_dtypes
import numpy as np

import concourse.bass as bass  # noqa: F401  (AP types resolve through this import)
import concourse.mybir as mybir
import concourse.tile as tile
from concourse import bacc, bass_utils

F32 = mybir.dt.float32
BF16 = mybir.dt.bfloat16
I16 = mybir.dt.int16
AT = mybir.ActivationFunctionType
ALU = mybir.AluOpType

B = 2
T = 2048
D = 1024
N_HEADS = 16
HEAD_DIM = 64
N_CORES = 8
S = 256            # per-core slice of the model dim (4 heads)
NHL = 4            # heads per core
P = 128
DC = D // P        # 8 contraction chunks for the projections
TC = T // P        # 16 token chunks
QH = 1024          # q-columns per attention block
SCALE = 1.0 / np.sqrt(HEAD_DIM)

LOG2E = float(np.log2(np.e))
C_SCH = 0.0579
A_SCH = float(SCALE * 128.0 * LOG2E)
B_SCH = float(127.0 * 128.0 - 128.0 * C_SCH)
# exp engine per (chunk%4, n-half): A=ScalarE exp, D=DVE Schraudolph.
# (Pool cannot read PSUM and DMA cannot read PSUM either, so exp stays
# on the two PSUM-capable engines, 4/4 split.)
HALF_ASSIGN = {
    (0, 0): "A", (0, 1): "D",
    (1, 0): "A", (1, 1): "D",
    (2, 0): "D", (2, 1): "A",
    (3, 0): "A", (3, 1): "D",
}

_CACHE = {}
LAST_STATS = {}


def _build_program():
    nc = bacc.Bacc("TRN2", target_bir_lowering=False, debug=False)

    xq_d = nc.dram_tensor("xq", [P, DC, T], BF16, kind="ExternalInput").ap()
    xk_d = nc.dram_tensor("xk", [P, DC, T], BF16, kind="ExternalInput").ap()
    xv_d = nc.dram_tensor("xv", [P, DC, T], BF16, kind="ExternalInput").ap()
    wqt_d = nc.dram_tensor("wqt", [P, DC, S], BF16, kind="ExternalInput").ap()
    wkt_d = nc.dram_tensor("wkt", [P, DC, S], BF16, kind="ExternalInput").ap()
    wvt_d = nc.dram_tensor("wvt", [P, DC, S], BF16, kind="ExternalInput").ap()
    bq_d = nc.dram_tensor("bq", [P, 2], F32, kind="ExternalInput").ap()
    bk_d = nc.dram_tensor("bk", [P, 2], F32, kind="ExternalInput").ap()
    bv_d = nc.dram_tensor("bv", [P, NHL, HEAD_DIM], F32, kind="ExternalInput").ap()
    wot_d = nc.dram_tensor("wot", [P, 2, D], BF16, kind="ExternalInput").ap()
    out_d = nc.dram_tensor("out_part", [TC, P, D], BF16, kind="ExternalOutput").ap()
    sums_d = nc.dram_tensor("sums_scr", [NHL, T], F32).ap()
    rsums_d = nc.dram_tensor("rsums_scr", [NHL, T], F32).ap()

    with tile.TileContext(nc) as tc:
        _body(tc, xq_d, xk_d, xv_d, wqt_d, wkt_d, wvt_d,
              bq_d, bk_d, bv_d, wot_d, out_d, sums_d, rsums_d)
    nc.compile()
    return nc


def _body(tc, xq_d, xk_d, xv_d, wqt_d, wkt_d, wvt_d, bq_d, bk_d, bv_d, wot_d,
          out_d, sums_d, rsums_d):
    nc = tc.nc

    from contextlib import ExitStack
    with ExitStack() as ctx:
        pers = ctx.enter_context(tc.tile_pool(name="pers", bufs=1))
        # per-head K^T/Q^T duplicated into both 64-partition halves
        kt2 = pers.tile([P, NHL, T], BF16, tag="kt2")
        qt2 = pers.tile([P, NHL, T], BF16, tag="qt2")
        v_sb = pers.tile([P, TC, NHL, 68], BF16, tag="v")
        attn_sb = pers.tile([P, 2, T], BF16, tag="attn")
        wot_sb = pers.tile([P, 2, D], BF16, tag="wot")
        bq_sb = pers.tile([P, 2], F32, tag="bq")
        bk_sb = pers.tile([P, 2], F32, tag="bk")
        bv_sb = pers.tile([P, NHL, HEAD_DIM], F32, tag="bv")
        zero_sb = pers.tile([P, 1], F32, tag="zero")
        scr_sb = pers.tile([P, 1], F32, tag="scr")

        nc.vector.memset(zero_sb[:], 0.0)
        # Preload the exp table set (~1.3us) before the attention pipeline.
        nc.scalar.activation(scr_sb[:], zero_sb[:], AT.Exp,
                             bias=zero_sb[:, 0:1], scale=1.0)
        # ones column for the P~V sum trick (col 64 of every (tchunk, head))
        nc.vector.memset(v_sb[:, :, :, 64:65], 1.0)

        # ---------------- Phase A: projections ----------------
        with tc.tile_pool(name="xw", bufs=1) as xw, \
             tc.tile_pool(name="kqsb", bufs=1) as kqsb, \
             tc.tile_pool(name="pjps", bufs=4, space="PSUM") as pjps:
            wq_sb = xw.tile([P, DC, S], BF16, tag="wq")
            wk_sb = xw.tile([P, DC, S], BF16, tag="wk")
            wv_sb = xw.tile([P, DC, S], BF16, tag="wv")
            xq_sb = xw.tile([P, DC, T], BF16, tag="xq")
            xk_sb = xw.tile([P, DC, T], BF16, tag="xk")
            xv_sb = xw.tile([P, DC, T], BF16, tag="xv")
            kt_sb = kqsb.tile([P, 2, T], BF16, tag="kt")
            qt_sb = kqsb.tile([P, 2, T], BF16, tag="qt")

            # weights in single DMAs; x in per-chunk DMAs so the d8-outer
            # accumulation can start as soon as chunk 0 lands
            nc.sync.dma_start(wk_sb[:], wkt_d[:])
            nc.sync.dma_start(bk_sb[:], bk_d[:])
            for c in range(DC):
                nc.sync.dma_start(xk_sb[:, c], xk_d[:, c])
            nc.sync.dma_start(wq_sb[:], wqt_d[:])
            nc.sync.dma_start(bq_sb[:], bq_d[:])
            for c in range(DC):
                nc.sync.dma_start(xq_sb[:, c], xq_d[:, c])
            nc.sync.dma_start(wv_sb[:], wvt_d[:])
            nc.sync.dma_start(bv_sb[:], bv_d[:])
            for c in range(DC):
                nc.sync.dma_start(xv_sb[:, c], xv_d[:, c])
            nc.sync.dma_start(wot_sb[:], wot_d[:])

            # K^T / Q^T projections: (256, T) d-major, bf16 + bias.
            def proj_kq(x_sb, w_sb, b_sb, dst, m):
                for n in range(4):
                    ps = pjps.tile([P, 512], F32, tag="pj")
                    for d8 in range(DC):
                        nc.tensor.matmul(
                            ps[:],
                            lhsT=w_sb[:, d8, m * P:(m + 1) * P],
                            rhs=x_sb[:, d8, n * 512:(n + 1) * 512],
                            start=(d8 == 0), stop=(d8 == DC - 1))
                    nc.vector.tensor_scalar(
                        dst[:, m, n * 512:(n + 1) * 512], ps[:],
                        b_sb[:, m:m + 1], None, op0=ALU.add)

            def dup_heads(src, dst, mh):
                # duplicate each head's 64 dims into both partition halves
                for hb in range(2):
                    h = 2 * mh + hb
                    s = src[hb * 64:(hb + 1) * 64, mh, :]
                    nc.sync.dma_start(dst[0:64, h, :], s)
                    nc.sync.dma_start(dst[64:128, h, :], s)

            proj_kq(xk_sb, wk_sb, bk_sb, kt_sb, 0)
            proj_kq(xq_sb, wq_sb, bq_sb, qt_sb, 0)
            dup_heads(kt_sb, kt2, 0)
            dup_heads(qt_sb, qt2, 0)
            proj_kq(xk_sb, wk_sb, bk_sb, kt_sb, 1)
            proj_kq(xq_sb, wq_sb, bq_sb, qt_sb, 1)
            dup_heads(kt_sb, kt2, 1)
            dup_heads(qt_sb, qt2, 1)

            # V projection: token-major (T, 256) bf16 + bias
            for t16 in range(TC):
                ps = pjps.tile([P, S], F32, tag="pj")
                for d8 in range(DC):
                    nc.tensor.matmul(
                        ps[:],
                        lhsT=xv_sb[:, d8, t16 * P:(t16 + 1) * P],
                        rhs=wv_sb[:, d8, :],
                        start=(d8 == 0), stop=(d8 == DC - 1))
                nc.vector.tensor_tensor(
                    v_sb[:, t16, :, 0:64],
                    ps[:].rearrange("p (h x) -> p h x", h=NHL),
                    bv_sb[:], op=ALU.add)

        # ---------------- Phase B: attention ----------------
        # 8 blocks (head, q-half), entirely in 64-row PE mode. Key chunks
        # are processed in pairs (i, i+1): chunk i on row tile T0
        # (partitions 0-63), chunk i+1 on T8 (64-127); consecutive
        # instructions always alternate tiles so LDWEIGHTS pulls ahead and
        # the two streams run concurrently.
        with tc.tile_pool(name="stp", bufs=4, space="PSUM") as stp, \
             tc.tile_pool(name="attlp", bufs=1, space="PSUM") as attlp, \
             tc.tile_pool(name="atthp", bufs=1, space="PSUM") as atthp, \
             tc.tile_pool(name="ptp", bufs=6) as ptp, \
             tc.tile_pool(name="shp", bufs=4) as shp, \
             tc.tile_pool(name="mrg", bufs=2) as mrgp, \
             tc.tile_pool(name="brd", bufs=2) as brdp, \
             tc.tile_pool(name="rcp", bufs=4) as rcpp:
            pending_pv = []

            def emit_pv(ent):
                att_lo, att_hi, h, i, pt = ent
                first = (i == 0)
                last = (i == TC - 1)
                for n in range(2):
                    ns = slice(n * 512, (n + 1) * 512)
                    nc.tensor.matmul(
                        att_lo[:, ns], lhsT=v_sb[0:64, i, h, 0:65],
                        rhs=pt[0:64, ns], start=first, stop=last)
                    nc.tensor.matmul(
                        att_hi[:, ns], lhsT=v_sb[64:128, i, h, 0:65],
                        rhs=pt[64:128, ns], start=first, stop=last)

            def emit_block_tail(ent):
                att_lo, att_hi, h, jh = ent
                q0 = jh * QH
                mh, hb = divmod(h, 2)
                ph = hb * 64
                # merge key-halves (also frees both PSUM accumulators);
                # only one tensor_tensor input may come from PSUM, so
                # copy att_lo out first (on ScalarE - DVE is exp-loaded).
                attm = mrgp.tile([65, QH], F32, tag="attm")
                nc.scalar.copy(attm[:], att_lo[:])
                nc.vector.tensor_tensor(attm[:], attm[:], att_hi[:],
                                        op=ALU.add)
                # softmax denominators -> reciprocal via DRAM bounce
                # (partition-major reciprocal, then partition broadcast)
                nc.sync.dma_start(sums_d[h:h + 1, q0:q0 + QH],
                                  attm[64:65, :])
                sp = rcpp.tile([P, QH // P], F32, tag="sp")
                nc.sync.dma_start(
                    sp[:], sums_d[h, q0:q0 + QH].rearrange(
                        "(p f) -> p f", p=P))
                rp = rcpp.tile([P, QH // P], F32, tag="rp")
                nc.vector.reciprocal(rp[:], sp[:])
                nc.sync.dma_start(
                    rsums_d[h, q0:q0 + QH].rearrange("(p f) -> p f", p=P),
                    rp[:])
                rc = brdp.tile([64, QH], F32, tag="rc")
                nc.sync.dma_start(
                    rc[:], rsums_d[h:h + 1, q0:q0 + QH].broadcast_to((64, QH)))
                # normalize on the (otherwise idle) Pool engine - all SBUF
                nc.gpsimd.tensor_tensor(
                    attn_sb[ph:ph + 64, mh, q0:q0 + QH],
                    attm[0:64, :], rc[:], op=ALU.mult)

            pending_tail = None
            blocks = [(h, jh) for h in range(NHL) for jh in range(2)]
            for bi, (h, jh) in enumerate(blocks):
                q0 = jh * QH
                att_lo = attlp.tile([65, QH], F32, tag="attl", name="att_lo")
                att_hi = atthp.tile([65, QH], F32, tag="atth", name="att_hi")
                for ip in range(TC // 2):
                    i0, i1 = 2 * ip, 2 * ip + 1
                    # st as four [P, 512] half-tiles: frees PSUM at half
                    # granularity, doubling the QK->exp pipeline depth
                    sts = {}
                    # interleave T0/T8 so pairs run concurrently
                    for n in range(2):
                        qs = slice(q0 + n * 512, q0 + (n + 1) * 512)
                        st0h = stp.tile([P, 512], F32, tag="st",
                                        name=f"st0n{n}")
                        st1h = stp.tile([P, 512], F32, tag="st",
                                        name=f"st1n{n}")
                        sts[(0, n)] = st0h
                        sts[(1, n)] = st1h
                        nc.tensor.matmul(
                            st0h[:],
                            lhsT=kt2[0:64, h, i0 * P:(i0 + 1) * P],
                            rhs=qt2[0:64, h, qs],
                            start=True, stop=True)
                        nc.tensor.matmul(
                            st1h[:],
                            lhsT=kt2[64:128, h, i1 * P:(i1 + 1) * P],
                            rhs=qt2[64:128, h, qs],
                            start=True, stop=True)
                    if bi == 0 and ip == 0:
                        # keep-warm (64-row shaped): bridge the exp
                        # pipeline-fill so HAM never sees a >3.4us PE idle.
                        for wmm in range(6):
                            nc.tensor.matmul(
                                att_lo[:, 0:512],
                                lhsT=v_sb[0:64, 0, h, 0:65],
                                rhs=qt2[0:64, h, 0:512],
                                start=True, stop=True)
                            nc.tensor.matmul(
                                att_hi[:, 0:512],
                                lhsT=v_sb[64:128, 0, h, 0:65],
                                rhs=qt2[64:128, h, 0:512],
                                start=True, stop=True)
                    pt0 = ptp.tile([P, QH], BF16, tag="pt", name="pt0")
                    pt1 = ptp.tile([P, QH], BF16, tag="pt", name="pt1")
                    # n0 halves first (their QK finishes first)
                    for n in range(2):
                        ns = slice(n * 512, (n + 1) * 512)
                        for ic, pt in ((0, pt0), (1, pt1)):
                            i = i0 + ic
                            st = sts[(ic, n)]
                            eng = HALF_ASSIGN[(i % 4, n)]
                            if eng == "A":
                                nc.scalar.activation(
                                    pt[:, ns], st[:], AT.Exp,
                                    bias=zero_sb[:, 0:1],
                                    scale=float(SCALE))
                            else:
                                nc.vector.tensor_scalar(
                                    pt[:, ns].bitcast(I16), st[:],
                                    A_SCH, B_SCH,
                                    op0=ALU.mult, op1=ALU.add)
                    pending_pv.append((att_lo, att_hi, h, i0, pt0))
                    pending_pv.append((att_lo, att_hi, h, i1, pt1))
                    while len(pending_pv) > 2:
                        emit_pv(pending_pv.pop(0))
                    if ip == 1 and pending_tail is not None:
                        emit_block_tail(pending_tail)
                        pending_tail = None
                pending_tail = (att_lo, att_hi, h, jh)
            while pending_pv:
                emit_pv(pending_pv.pop(0))
            emit_block_tail(pending_tail)

        # ---------------- Phase C: output projection (partial) ----------------
        with tc.tile_pool(name="ops", bufs=3, space="PSUM") as ops, \
             tc.tile_pool(name="owm", bufs=1, space="PSUM") as owm, \
             tc.tile_pool(name="osb", bufs=4) as osb:
            wmt = owm.tile([P, 512], F32, tag="wmt")
            for m in range(TC):
                po = ops.tile([P, D], F32, tag="po")
                # keep-warm matmul: evac-paced pipeline has PE gaps
                nc.tensor.matmul(wmt[:], lhsT=attn_sb[:, 0, 0:P],
                                 rhs=wot_sb[:, 0, 0:512], start=True, stop=True)
                for sc in range(2):
                    for n in range(2):
                        nc.tensor.matmul(
                            po[:, n * 512:(n + 1) * 512],
                            lhsT=attn_sb[:, sc, m * P:(m + 1) * P],
                            rhs=wot_sb[:, sc, n * 512:(n + 1) * 512],
                            start=(sc == 0), stop=(sc == 1))
                ob = osb.tile([P, D], BF16, tag="ob")
                if m % 2 == 0:
                    nc.scalar.copy(ob[:], po[:])
                else:
                    nc.vector.tensor_copy(ob[:], po[:])
                nc.sync.dma_start(out_d[m], ob[:])


def _shard_inputs(query, key, value, wq, bq, wk, bk, wv, bv, wo):
    """Build the 8 per-core input maps (all host-side numpy)."""
    bf16 = ml_dtypes.bfloat16
    in_maps = []

    def fold_dmajor(a_t, inner):
        # (D, inner) -> [P, DC, inner]
        return np.ascontiguousarray(
            a_t.reshape(DC, P, inner).transpose(1, 0, 2))

    xs = {}
    for b in range(B):
        for name, x in (("xq", query), ("xk", key), ("xv", value)):
            xt = np.ascontiguousarray(x[b].T).astype(bf16)  # (D, T)
            xs[(name, b)] = fold_dmajor(xt, T)

    for c in range(N_CORES):
        b, g = divmod(c, NHL)
        gs = g * S
        wq_g = wq[gs:gs + S]          # (S, D)
        wk_g = wk[gs:gs + S]
        wv_g = wv[gs:gs + S]
        wo_g = wo[:, gs:gs + S]       # (D, S)
        m = {
            "xq": xs[("xq", b)],
            "xk": xs[("xk", b)],
            "xv": xs[("xv", b)],
            "wqt": fold_dmajor(np.ascontiguousarray(wq_g.T).astype(bf16), S),
            "wkt": fold_dmajor(np.ascontiguousarray(wk_g.T).astype(bf16), S),
            "wvt": fold_dmajor(np.ascontiguousarray(wv_g.T).astype(bf16), S),
            "bq": np.ascontiguousarray(
                bq[gs:gs + S].reshape(2, P).T).astype(np.float32),
            "bk": np.ascontiguousarray(
                bk[gs:gs + S].reshape(2, P).T).astype(np.float32),
            "bv": np.ascontiguousarray(np.broadcast_to(
                bv[gs:gs + S].reshape(NHL, HEAD_DIM), (P, NHL, HEAD_DIM))
            ).astype(np.float32),
            "wot": np.ascontiguousarray(
                wo_g.T.reshape(2, P, D).transpose(1, 0, 2)).astype(bf16),
        }
        in_maps.append(m)
    return in_maps


def _reference_numpy(query, key, value, mask, wq, bq, wk, bk, wv, bv, wo, bo):
    """Pure-numpy fallback for non-trivial masks (never hit for spec inputs)."""
    def lin(x, w, b):
        return np.einsum("btd,od->bto", x, w) + b
    Bq, Tq, _ = query.shape
    Q = lin(query, wq, bq).reshape(Bq, Tq, N_HEADS, HEAD_DIM).transpose(0, 2, 1, 3)
    K = lin(key, wk, bk).reshape(Bq, Tq, N_HEADS, HEAD_DIM).transpose(0, 2, 1, 3)
    V = lin(value, wv, bv).reshape(Bq, Tq, N_HEADS, HEAD_DIM).transpose(0, 2, 1, 3)
    scores = np.einsum("bhqd,bhkd->bhqk", Q, K) * SCALE
    scores = np.where(mask[:, None, :, :] == 0, -np.inf, scores)
    scores = scores - scores.max(axis=-1, keepdims=True)
    e = np.exp(scores)
    probs = e / e.sum(axis=-1, keepdims=True)
    att = np.einsum("bhqk,bhkd->bhqd", probs, V)
    att = att.transpose(0, 2, 1, 3).reshape(Bq, Tq, N_HEADS * HEAD_DIM)
    return (np.einsum("btd,od->bto", att, wo) + bo).astype(np.float32)


def _enable_local_tracing():
    """Make bass_utils' axon NTFF-trace path work in this container."""
    import sys
    import types
    try:
        import antenv.axon_hooks  # noqa: F401
    except Exception:
        try:
            from trn_agent_boot.trn_boot import _ntff_profile_via_ctypes
            hook = _ntff_profile_via_ctypes("/opt/axon/libaxon_pjrt.so")
            if hook is None:
                return False
            holder = {"hook": hook}
            m2 = types.ModuleType("antenv.axon_hooks")
            m2.get_axon_ntff_profile_hook = lambda: holder["hook"]
            m2.set_axon_ntff_profile_hook = lambda h: holder.update(hook=h)
            if "antenv" not in sys.modules:
                m1 = types.ModuleType("antenv")
                m1.axon_hooks = m2
                sys.modules["antenv"] = m1
            else:
                sys.modules["antenv"].axon_hooks = m2
            sys.modules["antenv.axon_hooks"] = m2
        except Exception:
            return False
    bass_utils.upload_artifacts = lambda tmpdir: tmpdir
    return True


def kernel(query, key, value, mask, wq, bq, wk, bk, wv, bv, wo, bo):
    query = np.asarray(query, np.float32)
    key = np.asarray(key, np.float32)
    value = np.asarray(value, np.float32)
    wq_, bq_ = np.asarray(wq, np.float32), np.asarray(bq, np.float32)
    wk_, bk_ = np.asarray(wk, np.float32), np.asarray(bk, np.float32)
    wv_, bv_ = np.asarray(wv, np.float32), np.asarray(bv, np.float32)
    wo_, bo_ = np.asarray(wo, np.float32), np.asarray(bo, np.float32)
    mask_np = np.asarray(mask)

    if not np.all(mask_np != 0):
        # Spec inputs always have an all-ones mask; keep a correct fallback.
        return _reference_numpy(query, key, value, mask_np, wq_, bq_,
                                wk_, bk_, wv_, bv_, wo_, bo_)

    if "prog" not in _CACHE:
        _CACHE["prog"] = _build_program()
    nc = _CACHE["prog"]

    in_maps = _shard_inputs(query, key, value, wq_, bq_, wk_, bk_, wv_, bv_, wo_)

    trace = os.environ.get("KERNEL_TRACE", "0") == "1"
    kw = {}
    if trace:
        trace = _enable_local_tracing()
        if trace:
            tdir = os.environ.get("KERNEL_TRACE_DIR")
            if tdir:
                os.makedirs(tdir, exist_ok=True)
                kw["tmpdir"] = tdir
    try:
        res = bass_utils.run_bass_kernel_spmd(
            nc, in_maps, core_ids=list(range(N_CORES)), trace=trace, **kw)
    except Exception:
        if not trace:
            raise
        import traceback
        traceback.print_exc()
        res = bass_utils.run_bass_kernel_spmd(
            nc, in_maps, core_ids=list(range(N_CORES)), trace=False)

    LAST_STATS.clear()
    LAST_STATS["exec_time_ns"] = res.exec_time_ns
    LAST_STATS["profile_json"] = res.profile_json
    if res.instructions_and_trace is not None:
        LAST_STATS["trace_url"] = res.instructions_and_trace[1]

    out = np.empty((B, T, D), np.float32)
    for b in range(B):
        acc = np.zeros((T, D), np.float32)
        for g in range(NHL):
            acc += res.results[b * NHL + g]["out_part"].reshape(T, D).astype(
                np.float32)
        out[b] = acc + bo_
    return out


# revision 15
# speedup vs baseline: 1.1656x; 1.0265x over previous
"""Multi-head attention (COAMultiHeadAttention) on 8 Trainium2 NeuronCores.

Sharding: batch x head-group. Core c (0..7) handles batch b = c//4 and head
group g = c%4 (4 of 16 heads, i.e. a 256-wide slice of the 1024-dim model).

Per-core schedule (v2 - row-tiled attention):
  Phase A: q/k/v projections in bf16 (d-chunk-outer PSUM accumulation),
    evac with bias add on DVE. K^T/Q^T are then DUPLICATED into both
    64-partition halves per head (SBUF->SBUF DMA) so phase B can run the
    PE as two independent 64x128 row tiles.
  Phase B: per (head, q-half) block, entirely in 64-row PE mode (no
    mode switches - mixing 64/128-row matmuls costs ~370ns per switch):
      - QK^T for key chunks i, i+1 run CONCURRENTLY on row tiles T0/T8
        (measured 245ns per pair vs 430ns serial).
      - exp is split across ScalarE (table exp) and DVE (one-instruction
        Schraudolph exp: bits16 = x*a+b written to an int16 view, bitcast
        bf16). DVE share ~6/16 keeps the approx error ~1% (softmax
        denominators use the same approximated probs, so the mean error
        cancels).
      - P~V with an extra ones column runs as key-half pairs on T0/T8
        into separate PSUM accumulators att_lo/att_hi (a single PSUM
        accumulation group must stay on one row tile - crossing tiles
        hangs the device).
      - Tail: att_lo+att_hi merge (DVE), softmax denominators from the
        ones column, reciprocal broadcast via DRAM bounce, normalize.
  Phase C: output projection partials in 128-row mode, evac alternating
    ScalarE/DVE. Host sums the 4 partials per batch in fp32 and adds bo.
"""

import os

import ml_dtypes
import numpy as np

import concourse.bass as bass  # noqa: F401  (AP types resolve through this import)
import concourse.mybir as mybir
import concourse.tile as tile
from concourse import bacc, bass_utils

F32 = mybir.dt.float32
BF16 = mybir.dt.bfloat16
I16 = mybir.dt.int16
AT = mybir.ActivationFunctionType
ALU = mybir.AluOpType

B = 2
T = 2048
D = 1024
N_HEADS = 16
HEAD_DIM = 64
N_CORES = 8
S = 256            # per-core slice of the model dim (4 heads)
NHL = 4            # heads per core
P = 128
DC = D // P        # 8 contraction chunks for the projections
TC = T // P        # 16 token chunks
QH = 1024          # q-columns per attention block
SCALE = 1.0 / np.sqrt(HEAD_DIM)

LOG2E = float(np.log2(np.e))
C_SCH = 0.0579
A_SCH = float(SCALE * 128.0 * LOG2E)
B_SCH = float(127.0 * 128.0 - 128.0 * C_SCH)
# exp engine per (chunk%4, n-half): A=ScalarE exp, D=DVE Schraudolph.
# (Pool cannot read PSUM and DMA cannot read PSUM either, so exp stays
# on the two PSUM-capable engines, 4/4 split.)
HALF_ASSIGN = {
    (0, 0): "A", (0, 1): "D",
    (1, 0): "A", (1, 1): "D",
    (2, 0): "D", (2, 1): "A",
    (3, 0): "A", (3, 1): "D",
}

_CACHE = {}
LAST_STATS = {}


def _build_program():
    nc = bacc.Bacc("TRN2", target_bir_lowering=False, debug=False)

    xq_d = nc.dram_tensor("xq", [P, DC, T], BF16, kind="ExternalInput").ap()
    xk_d = nc.dram_tensor("xk", [P, DC, T], BF16, kind="ExternalInput").ap()
    xv_d = nc.dram_tensor("xv", [P, DC, T], BF16, kind="ExternalInput").ap()
    wqt_d = nc.dram_tensor("wqt", [P, DC, S], BF16, kind="ExternalInput").ap()
    wkt_d = nc.dram_tensor("wkt", [P, DC, S], BF16, kind="ExternalInput").ap()
    wvt_d = nc.dram_tensor("wvt", [P, DC, S], BF16, kind="ExternalInput").ap()
    bq_d = nc.dram_tensor("bq", [P, 2], F32, kind="ExternalInput").ap()
    bk_d = nc.dram_tensor("bk", [P, 2], F32, kind="ExternalInput").ap()
    bv_d = nc.dram_tensor("bv", [P, NHL, HEAD_DIM], F32, kind="ExternalInput").ap()
    wot_d = nc.dram_tensor("wot", [P, 2, D], BF16, kind="ExternalInput").ap()
    out_d = nc.dram_tensor("out_part", [TC, P, D], BF16, kind="ExternalOutput").ap()
    sums_d = nc.dram_tensor("sums_scr", [NHL, T], F32).ap()
    rsums_d = nc.dram_tensor("rsums_scr", [NHL, T], F32).ap()

    with tile.TileContext(nc) as tc:
        _body(tc, xq_d, xk_d, xv_d, wqt_d, wkt_d, wvt_d,
              bq_d, bk_d, bv_d, wot_d, out_d, sums_d, rsums_d)
    nc.compile()
    return nc


def _body(tc, xq_d, xk_d, xv_d, wqt_d, wkt_d, wvt_d, bq_d, bk_d, bv_d, wot_d,
          out_d, sums_d, rsums_d):
    nc = tc.nc

    from contextlib import ExitStack
    with ExitStack() as ctx:
        pers = ctx.enter_context(tc.tile_pool(name="pers", bufs=1))
        # per-head K^T/Q^T duplicated into both 64-partition halves
        kt2 = pers.tile([P, NHL, T], BF16, tag="kt2")
        qt2 = pers.tile([P, NHL, T], BF16, tag="qt2")
        v_sb = pers.tile([P, TC, NHL, 68], BF16, tag="v")
        attn_sb = pers.tile([P, 2, T], BF16, tag="attn")
        wot_sb = pers.tile([P, 2, D], BF16, tag="wot")
        bq_sb = pers.tile([P, 2], F32, tag="bq")
        bk_sb = pers.tile([P, 2], F32, tag="bk")
        bv_sb = pers.tile([P, NHL, HEAD_DIM], F32, tag="bv")
        zero_sb = pers.tile([P, 1], F32, tag="zero")
        scr_sb = pers.tile([P, 1], F32, tag="scr")

        nc.vector.memset(zero_sb[:], 0.0)
        # Preload the exp table set (~1.3us) before the attention pipeline.
        nc.scalar.activation(scr_sb[:], zero_sb[:], AT.Exp,
                             bias=zero_sb[:, 0:1], scale=1.0)
        # ones column for the P~V sum trick (col 64 of every (tchunk, head))
        nc.vector.memset(v_sb[:, :, :, 64:65], 1.0)

        # ---------------- Phase A: projections ----------------
        with tc.tile_pool(name="xw", bufs=1) as xw, \
             tc.tile_pool(name="kqsb", bufs=1) as kqsb, \
             tc.tile_pool(name="pjps", bufs=4, space="PSUM") as pjps:
            wq_sb = xw.tile([P, DC, S], BF16, tag="wq")
            wk_sb = xw.tile([P, DC, S], BF16, tag="wk")
            wv_sb = xw.tile([P, DC, S], BF16, tag="wv")
            xq_sb = xw.tile([P, DC, T], BF16, tag="xq")
            xk_sb = xw.tile([P, DC, T], BF16, tag="xk")
            xv_sb = xw.tile([P, DC, T], BF16, tag="xv")
            kt_sb = kqsb.tile([P, 2, T], BF16, tag="kt")
            qt_sb = kqsb.tile([P, 2, T], BF16, tag="qt")

            # weights in single DMAs; x in per-chunk DMAs so the d8-outer
            # accumulation can start as soon as chunk 0 lands
            # halves of each x tensor per DMA: large contiguous
            # per-partition runs (16KB) instead of 4KB-descriptor chunks,
            # while still letting the d8-outer accumulation start early
            nc.sync.dma_start(wk_sb[:], wkt_d[:])
            nc.sync.dma_start(bk_sb[:], bk_d[:])
            nc.sync.dma_start(xk_sb[:, 0:4], xk_d[:, 0:4])
            nc.sync.dma_start(xk_sb[:, 4:8], xk_d[:, 4:8])
            nc.sync.dma_start(wq_sb[:], wqt_d[:])
            nc.sync.dma_start(bq_sb[:], bq_d[:])
            nc.sync.dma_start(xq_sb[:, 0:4], xq_d[:, 0:4])
            nc.sync.dma_start(xq_sb[:, 4:8], xq_d[:, 4:8])
            nc.sync.dma_start(wv_sb[:], wvt_d[:])
            nc.sync.dma_start(bv_sb[:], bv_d[:])
            nc.sync.dma_start(xv_sb[:, 0:4], xv_d[:, 0:4])
            nc.sync.dma_start(xv_sb[:, 4:8], xv_d[:, 4:8])
            nc.sync.dma_start(wot_sb[:], wot_d[:])

            # K^T / Q^T projections: (256, T) d-major, bf16 + bias.
            def proj_kq(x_sb, w_sb, b_sb, dst, m):
                for n in range(4):
                    ps = pjps.tile([P, 512], F32, tag="pj")
                    for d8 in range(DC):
                        nc.tensor.matmul(
                            ps[:],
                            lhsT=w_sb[:, d8, m * P:(m + 1) * P],
                            rhs=x_sb[:, d8, n * 512:(n + 1) * 512],
                            start=(d8 == 0), stop=(d8 == DC - 1))
                    nc.vector.tensor_scalar(
                        dst[:, m, n * 512:(n + 1) * 512], ps[:],
                        b_sb[:, m:m + 1], None, op0=ALU.add)

            def dup_heads(src, dst, mh):
                # duplicate each head's 64 dims into both partition halves
                for hb in range(2):
                    h = 2 * mh + hb
                    s = src[hb * 64:(hb + 1) * 64, mh, :]
                    nc.sync.dma_start(dst[0:64, h, :], s)
                    nc.sync.dma_start(dst[64:128, h, :], s)

            proj_kq(xk_sb, wk_sb, bk_sb, kt_sb, 0)
            proj_kq(xq_sb, wq_sb, bq_sb, qt_sb, 0)
            dup_heads(kt_sb, kt2, 0)
            dup_heads(qt_sb, qt2, 0)
            proj_kq(xk_sb, wk_sb, bk_sb, kt_sb, 1)
            proj_kq(xq_sb, wq_sb, bq_sb, qt_sb, 1)
            dup_heads(kt_sb, kt2, 1)
            dup_heads(qt_sb, qt2, 1)

            # V projection: token-major (T, 256) bf16 + bias
            for t16 in range(TC):
                ps = pjps.tile([P, S], F32, tag="pj")
                for d8 in range(DC):
                    nc.tensor.matmul(
                        ps[:],
                        lhsT=xv_sb[:, d8, t16 * P:(t16 + 1) * P],
                        rhs=wv_sb[:, d8, :],
                        start=(d8 == 0), stop=(d8 == DC - 1))
                nc.vector.tensor_tensor(
                    v_sb[:, t16, :, 0:64],
                    ps[:].rearrange("p (h x) -> p h x", h=NHL),
                    bv_sb[:], op=ALU.add)

        # ---------------- Phase B: attention ----------------
        # 8 blocks (head, q-half), entirely in 64-row PE mode. Key chunks
        # are processed in pairs (i, i+1): chunk i on row tile T0
        # (partitions 0-63), chunk i+1 on T8 (64-127); consecutive
        # instructions always alternate tiles so LDWEIGHTS pulls ahead and
        # the two streams run concurrently.
        with tc.tile_pool(name="stp", bufs=4, space="PSUM") as stp, \
             tc.tile_pool(name="attlp", bufs=1, space="PSUM") as attlp, \
             tc.tile_pool(name="atthp", bufs=1, space="PSUM") as atthp, \
             tc.tile_pool(name="ptp", bufs=6) as ptp, \
             tc.tile_pool(name="shp", bufs=4) as shp, \
             tc.tile_pool(name="mrg", bufs=2) as mrgp, \
             tc.tile_pool(name="brd", bufs=2) as brdp, \
             tc.tile_pool(name="rcp", bufs=4) as rcpp:
            pending_pv = []

            def emit_pv(ent):
                att_lo, att_hi, h, i, pt = ent
                first = (i == 0)
                last = (i == TC - 1)
                for n in range(2):
                    ns = slice(n * 512, (n + 1) * 512)
                    nc.tensor.matmul(
                        att_lo[:, ns], lhsT=v_sb[0:64, i, h, 0:65],
                        rhs=pt[0:64, ns], start=first, stop=last)
                    nc.tensor.matmul(
                        att_hi[:, ns], lhsT=v_sb[64:128, i, h, 0:65],
                        rhs=pt[64:128, ns], start=first, stop=last)

            def emit_block_tail(ent):
                att_lo, att_hi, h, jh = ent
                q0 = jh * QH
                mh, hb = divmod(h, 2)
                ph = hb * 64
                # merge key-halves (also frees both PSUM accumulators);
                # only one tensor_tensor input may come from PSUM, so
                # copy att_lo out first (on ScalarE - DVE is exp-loaded).
                attm = mrgp.tile([65, QH], F32, tag="attm")
                nc.scalar.copy(attm[:], att_lo[:])
                nc.vector.tensor_tensor(attm[:], attm[:], att_hi[:],
                                        op=ALU.add)
                # softmax denominators -> reciprocal via DRAM bounce
                # (partition-major reciprocal, then partition broadcast)
                nc.sync.dma_start(sums_d[h:h + 1, q0:q0 + QH],
                                  attm[64:65, :])
                sp = rcpp.tile([P, QH // P], F32, tag="sp")
                nc.sync.dma_start(
                    sp[:], sums_d[h, q0:q0 + QH].rearrange(
                        "(p f) -> p f", p=P))
                rp = rcpp.tile([P, QH // P], F32, tag="rp")
                nc.vector.reciprocal(rp[:], sp[:])
                nc.sync.dma_start(
                    rsums_d[h, q0:q0 + QH].rearrange("(p f) -> p f", p=P),
                    rp[:])
                rc = brdp.tile([64, QH], F32, tag="rc")
                nc.sync.dma_start(
                    rc[:], rsums_d[h:h + 1, q0:q0 + QH].broadcast_to((64, QH)))
                # normalize on the (otherwise idle) Pool engine - all SBUF
                nc.gpsimd.tensor_tensor(
                    attn_sb[ph:ph + 64, mh, q0:q0 + QH],
                    attm[0:64, :], rc[:], op=ALU.mult)

            pending_tail = None
            blocks = [(h, jh) for h in range(NHL) for jh in range(2)]
            for bi, (h, jh) in enumerate(blocks):
                q0 = jh * QH
                att_lo = attlp.tile([65, QH], F32, tag="attl", name="att_lo")
                att_hi = atthp.tile([65, QH], F32, tag="atth", name="att_hi")
                for ip in range(TC // 2):
                    i0, i1 = 2 * ip, 2 * ip + 1
                    # st as four [P, 512] half-tiles: frees PSUM at half
                    # granularity, doubling the QK->exp pipeline depth
                    sts = {}
                    # interleave T0/T8 so pairs run concurrently
                    for n in range(2):
                        qs = slice(q0 + n * 512, q0 + (n + 1) * 512)
                        st0h = stp.tile([P, 512], F32, tag="st",
                                        name=f"st0n{n}")
                        st1h = stp.tile([P, 512], F32, tag="st",
                                        name=f"st1n{n}")
                        sts[(0, n)] = st0h
                        sts[(1, n)] = st1h
                        nc.tensor.matmul(
                            st0h[:],
                            lhsT=kt2[0:64, h, i0 * P:(i0 + 1) * P],
                            rhs=qt2[0:64, h, qs],
                            start=True, stop=True)
                        nc.tensor.matmul(
                            st1h[:],
                            lhsT=kt2[64:128, h, i1 * P:(i1 + 1) * P],
                            rhs=qt2[64:128, h, qs],
                            start=True, stop=True)
                    if bi == 0 and ip == 0:
                        # keep-warm (64-row shaped): bridge the exp
                        # pipeline-fill so HAM never sees a >3.4us PE idle.
                        for wmm in range(6):
                            nc.tensor.matmul(
                                att_lo[:, 0:512],
                                lhsT=v_sb[0:64, 0, h, 0:65],
                                rhs=qt2[0:64, h, 0:512],
                                start=True, stop=True)
                            nc.tensor.matmul(
                                att_hi[:, 0:512],
                                lhsT=v_sb[64:128, 0, h, 0:65],
                                rhs=qt2[64:128, h, 0:512],
                                start=True, stop=True)
                    pt0 = ptp.tile([P, QH], BF16, tag="pt", name="pt0")
                    pt1 = ptp.tile([P, QH], BF16, tag="pt", name="pt1")
                    # n0 halves first (their QK finishes first)
                    for n in range(2):
                        ns = slice(n * 512, (n + 1) * 512)
                        for ic, pt in ((0, pt0), (1, pt1)):
                            i = i0 + ic
                            st = sts[(ic, n)]
                            eng = HALF_ASSIGN[(i % 4, n)]
                            if eng == "A":
                                nc.scalar.activation(
                                    pt[:, ns], st[:], AT.Exp,
                                    bias=zero_sb[:, 0:1],
                                    scale=float(SCALE))
                            else:
                                nc.vector.tensor_scalar(
                                    pt[:, ns].bitcast(I16), st[:],
                                    A_SCH, B_SCH,
                                    op0=ALU.mult, op1=ALU.add)
                    pending_pv.append((att_lo, att_hi, h, i0, pt0))
                    pending_pv.append((att_lo, att_hi, h, i1, pt1))
                    while len(pending_pv) > 2:
                        emit_pv(pending_pv.pop(0))
                    if ip == 1 and pending_tail is not None:
                        emit_block_tail(pending_tail)
                        pending_tail = None
                pending_tail = (att_lo, att_hi, h, jh)
            while pending_pv:
                emit_pv(pending_pv.pop(0))
            emit_block_tail(pending_tail)

        # ---------------- Phase C: output projection (partial) ----------------
        with tc.tile_pool(name="ops", bufs=3, space="PSUM") as ops, \
             tc.tile_pool(name="owm", bufs=1, space="PSUM") as owm, \
             tc.tile_pool(name="osb", bufs=4) as osb:
            wmt = owm.tile([P, 512], F32, tag="wmt")
            for m in range(TC):
                po = ops.tile([P, D], F32, tag="po")
                # keep-warm matmul: evac-paced pipeline has PE gaps
                nc.tensor.matmul(wmt[:], lhsT=attn_sb[:, 0, 0:P],
                                 rhs=wot_sb[:, 0, 0:512], start=True, stop=True)
                for sc in range(2):
                    for n in range(2):
                        nc.tensor.matmul(
                            po[:, n * 512:(n + 1) * 512],
                            lhsT=attn_sb[:, sc, m * P:(m + 1) * P],
                            rhs=wot_sb[:, sc, n * 512:(n + 1) * 512],
                            start=(sc == 0), stop=(sc == 1))
                ob = osb.tile([P, D], BF16, tag="ob")
                if m % 2 == 0:
                    nc.scalar.copy(ob[:], po[:])
                else:
                    nc.vector.tensor_copy(ob[:], po[:])
                nc.sync.dma_start(out_d[m], ob[:])


def _shard_inputs(query, key, value, wq, bq, wk, bk, wv, bv, wo):
    """Build the 8 per-core input maps (all host-side numpy)."""
    bf16 = ml_dtypes.bfloat16
    in_maps = []

    def fold_dmajor(a_t, inner):
        # (D, inner) -> [P, DC, inner]
        return np.ascontiguousarray(
            a_t.reshape(DC, P, inner).transpose(1, 0, 2))

    xs = {}
    for b in range(B):
        for name, x in (("xq", query), ("xk", key), ("xv", value)):
            xt = np.ascontiguousarray(x[b].T).astype(bf16)  # (D, T)
            xs[(name, b)] = fold_dmajor(xt, T)

    for c in range(N_CORES):
        b, g = divmod(c, NHL)
        gs = g * S
        wq_g = wq[gs:gs + S]          # (S, D)
        wk_g = wk[gs:gs + S]
        wv_g = wv[gs:gs + S]
        wo_g = wo[:, gs:gs + S]       # (D, S)
        m = {
            "xq": xs[("xq", b)],
            "xk": xs[("xk", b)],
            "xv": xs[("xv", b)],
            "wqt": fold_dmajor(np.ascontiguousarray(wq_g.T).astype(bf16), S),
            "wkt": fold_dmajor(np.ascontiguousarray(wk_g.T).astype(bf16), S),
            "wvt": fold_dmajor(np.ascontiguousarray(wv_g.T).astype(bf16), S),
            "bq": np.ascontiguousarray(
                bq[gs:gs + S].reshape(2, P).T).astype(np.float32),
            "bk": np.ascontiguousarray(
                bk[gs:gs + S].reshape(2, P).T).astype(np.float32),
            "bv": np.ascontiguousarray(np.broadcast_to(
                bv[gs:gs + S].reshape(NHL, HEAD_DIM), (P, NHL, HEAD_DIM))
            ).astype(np.float32),
            "wot": np.ascontiguousarray(
                wo_g.T.reshape(2, P, D).transpose(1, 0, 2)).astype(bf16),
        }
        in_maps.append(m)
    return in_maps


def _reference_numpy(query, key, value, mask, wq, bq, wk, bk, wv, bv, wo, bo):
    """Pure-numpy fallback for non-trivial masks (never hit for spec inputs)."""
    def lin(x, w, b):
        return np.einsum("btd,od->bto", x, w) + b
    Bq, Tq, _ = query.shape
    Q = lin(query, wq, bq).reshape(Bq, Tq, N_HEADS, HEAD_DIM).transpose(0, 2, 1, 3)
    K = lin(key, wk, bk).reshape(Bq, Tq, N_HEADS, HEAD_DIM).transpose(0, 2, 1, 3)
    V = lin(value, wv, bv).reshape(Bq, Tq, N_HEADS, HEAD_DIM).transpose(0, 2, 1, 3)
    scores = np.einsum("bhqd,bhkd->bhqk", Q, K) * SCALE
    scores = np.where(mask[:, None, :, :] == 0, -np.inf, scores)
    scores = scores - scores.max(axis=-1, keepdims=True)
    e = np.exp(scores)
    probs = e / e.sum(axis=-1, keepdims=True)
    att = np.einsum("bhqk,bhkd->bhqd", probs, V)
    att = att.transpose(0, 2, 1, 3).reshape(Bq, Tq, N_HEADS * HEAD_DIM)
    return (np.einsum("btd,od->bto", att, wo) + bo).astype(np.float32)


def _enable_local_tracing():
    """Make bass_utils' axon NTFF-trace path work in this container."""
    import sys
    import types
    try:
        import antenv.axon_hooks  # noqa: F401
    except Exception:
        try:
            from trn_agent_boot.trn_boot import _ntff_profile_via_ctypes
            hook = _ntff_profile_via_ctypes("/opt/axon/libaxon_pjrt.so")
            if hook is None:
                return False
            holder = {"hook": hook}
            m2 = types.ModuleType("antenv.axon_hooks")
            m2.get_axon_ntff_profile_hook = lambda: holder["hook"]
            m2.set_axon_ntff_profile_hook = lambda h: holder.update(hook=h)
            if "antenv" not in sys.modules:
                m1 = types.ModuleType("antenv")
                m1.axon_hooks = m2
                sys.modules["antenv"] = m1
            else:
                sys.modules["antenv"].axon_hooks = m2
            sys.modules["antenv.axon_hooks"] = m2
        except Exception:
            return False
    bass_utils.upload_artifacts = lambda tmpdir: tmpdir
    return True


def kernel(query, key, value, mask, wq, bq, wk, bk, wv, bv, wo, bo):
    query = np.asarray(query, np.float32)
    key = np.asarray(key, np.float32)
    value = np.asarray(value, np.float32)
    wq_, bq_ = np.asarray(wq, np.float32), np.asarray(bq, np.float32)
    wk_, bk_ = np.asarray(wk, np.float32), np.asarray(bk, np.float32)
    wv_, bv_ = np.asarray(wv, np.float32), np.asarray(bv, np.float32)
    wo_, bo_ = np.asarray(wo, np.float32), np.asarray(bo, np.float32)
    mask_np = np.asarray(mask)

    if not np.all(mask_np != 0):
        # Spec inputs always have an all-ones mask; keep a correct fallback.
        return _reference_numpy(query, key, value, mask_np, wq_, bq_,
                                wk_, bk_, wv_, bv_, wo_, bo_)

    if "prog" not in _CACHE:
        _CACHE["prog"] = _build_program()
    nc = _CACHE["prog"]

    in_maps = _shard_inputs(query, key, value, wq_, bq_, wk_, bk_, wv_, bv_, wo_)

    trace = os.environ.get("KERNEL_TRACE", "0") == "1"
    kw = {}
    if trace:
        trace = _enable_local_tracing()
        if trace:
            tdir = os.environ.get("KERNEL_TRACE_DIR")
            if tdir:
                os.makedirs(tdir, exist_ok=True)
                kw["tmpdir"] = tdir
    try:
        res = bass_utils.run_bass_kernel_spmd(
            nc, in_maps, core_ids=list(range(N_CORES)), trace=trace, **kw)
    except Exception:
        if not trace:
            raise
        import traceback
        traceback.print_exc()
        res = bass_utils.run_bass_kernel_spmd(
            nc, in_maps, core_ids=list(range(N_CORES)), trace=False)

    LAST_STATS.clear()
    LAST_STATS["exec_time_ns"] = res.exec_time_ns
    LAST_STATS["profile_json"] = res.profile_json
    if res.instructions_and_trace is not None:
        LAST_STATS["trace_url"] = res.instructions_and_trace[1]

    out = np.empty((B, T, D), np.float32)
    for b in range(B):
        acc = np.zeros((T, D), np.float32)
        for g in range(NHL):
            acc += res.results[b * NHL + g]["out_part"].reshape(T, D).astype(
                np.float32)
        out[b] = acc + bo_
    return out
